# revision 1
# baseline (speedup 1.0000x reference)
"""Self-contained TRN2 Bass kernel for causal self-attention (B=2,T=2048,D=1024,H=16).

kernel(**inputs) takes the full unsharded inputs and returns the full output.
Sharding: 8 NeuronCores; core c -> batch b=c//4, head-group g=c%4 (4 heads).
Each core runs projections + RoPE + causal flash-style attention (transposed
scores, deferred softmax normalization) + a partial output projection; the
host sums the 4 per-batch partials and adds the output bias.
"""

import math
from contextlib import ExitStack

import numpy as np

import concourse.bass as bass
import concourse.tile as tile
from concourse import bacc, mybir

F32 = mybir.dt.float32
F32R = mybir.dt.float32r
BF16 = mybir.dt.bfloat16

B, T, D, H, HD = 2, 2048, 1024, 16, 64
P = 128
KT = D // P            # 8 k-slabs for projections
NT = T // P            # 16 t/k tiles
QS = 512               # q-slab width for attention
NQS = T // QS          # 4 q-slabs
HPG = 4                # heads per core


class Cfg:
    mm_proj = F32R     # dtype for projection matmuls
    mm_attn = F32R     # dtype for scores/AV matmuls
    mm_out = F32R      # dtype for output projection


def _r(ap, dt_):
    return ap


def build_nc(cfg=Cfg, num_devices=8):
    nc = bacc.Bacc("TRN2", target_bir_lowering=False, debug=False,
                   num_devices=num_devices)
    ext = dict(kind="ExternalInput")
    xT = nc.dram_tensor("xT", [D, T], BF16, **ext).ap()
    wq = nc.dram_tensor("wq", [D, 2 * P], BF16, **ext).ap()
    wk = nc.dram_tensor("wk", [D, 2 * P], BF16, **ext).ap()
    wv = nc.dram_tensor("wv", [D, 2 * P], BF16, **ext).ap()
    wo = nc.dram_tensor("wo", [2 * P, D], F32R, **ext).ap()
    csc = nc.dram_tensor("csc", [P, T], BF16, **ext).ap()
    ssc = nc.dram_tensor("ssc", [P, T], BF16, **ext).ap()
    bq2 = nc.dram_tensor("bq2", [P, 2], F32, **ext).ap()
    bk2 = nc.dram_tensor("bk2", [P, 2], F32, **ext).ap()
    bvr = nc.dram_tensor("bvr", [1, 2 * P], BF16, **ext).ap()
    m0 = nc.dram_tensor("m0", [P, P], BF16, **ext).ap()
    ones_in = nc.dram_tensor("ones_in", [P, P], BF16, **ext).ap()
    out = nc.dram_tensor("out", [T, D], F32, kind="ExternalOutput").ap()

    with tile.TileContext(nc) as tc:
        _body(tc, cfg, xT, wq, wk, wv, wo, csc, ssc, bq2, bk2, bvr, m0,
              ones_in, out)
    nc.compile()
    return nc


def _body(tc, cfg, xT, wq, wk, wv, wo, csc, ssc, bq2, bk2, bvr, m0,
          ones_in, out):
    nc = tc.nc
    Copy = mybir.ActivationFunctionType.Copy
    Ln = mybir.ActivationFunctionType.Ln
    Exp = mybir.ActivationFunctionType.Exp
    Ident = mybir.ActivationFunctionType.Identity

    with ExitStack() as outer:
        consts = outer.enter_context(tc.tile_pool(name="consts", bufs=1))
        wpool = outer.enter_context(tc.tile_pool(name="w", bufs=2))
        qk = outer.enter_context(tc.tile_pool(name="qk", bufs=1))
        vp = outer.enter_context(tc.tile_pool(name="v", bufs=1))
        atp = outer.enter_context(tc.tile_pool(name="at", bufs=1))

        bq_s = consts.tile([P, 2], F32, tag="bq")
        bk_s = consts.tile([P, 2], F32, tag="bk")
        bv_s = consts.tile([1, 2 * P], BF16, tag="bv")
        nc.gpsimd.dma_start(bv_s[:], bvr)
        m0_s = consts.tile([P, P], BF16, tag="m0")
        nc.gpsimd.dma_start(m0_s[:], m0)
        ones_s = consts.tile([1, P], BF16, tag="ones")
        nc.gpsimd.dma_start(ones_s[:], ones_in[0:1, :])

        v_s = vp.tile([P, NT, HPG, HD + 1], BF16)
        nc.gpsimd.dma_start(
            v_s[:, :, :, HD:HD + 1],
            ones_in[:, 0:NT * HPG].rearrange("p (t h) -> p t h", t=NT)[:, :, :, None],
        )
        at0 = atp.tile([P, T], F32R, tag="at0")
        at1 = atp.tile([P, T], F32R, tag="at1")
        at_tiles = (at0, at1)

        # ================= phase A: projections + RoPE =================
        with ExitStack() as pha:
            xtp = pha.enter_context(tc.tile_pool(name="xt", bufs=1))
            cscp = pha.enter_context(tc.tile_pool(name="cs", bufs=1))
            rtmp = pha.enter_context(tc.tile_pool(name="rtmp", bufs=3))
            ps_proj = pha.enter_context(
                tc.tile_pool(name="psp", bufs=8, space="PSUM"))

            # weights first (small), then xT in per-slab chunks so the first
            # matmuls can start as soon as slab 0 lands
            wqs_list = []
            for wt in (wq, wk):
                w_s = wpool.tile([P, KT, 2 * P], BF16, tag="w")
                nc.scalar.dma_start(
                    w_s[:], wt.rearrange("(ko ki) m -> ki ko m", ki=P))
                wqs_list.append(w_s)
            nc.scalar.dma_start(bq_s[:], bq2)
            nc.scalar.dma_start(bk_s[:], bk2)
            xts = []
            xTr = xT.rearrange("(ko ki) t -> ki ko t", ki=P)
            for kt in range(KT):
                xc = xtp.tile([P, T], BF16, tag=f"xt{kt}")
                nc.sync.dma_start(xc[:], xTr[:, kt, :])
                xts.append(xc)
            csc_s = cscp.tile([P, T], BF16, tag="c")
            nc.scalar.dma_start(csc_s[:], csc)
            ssc_s = cscp.tile([P, T], BF16, tag="s")
            nc.scalar.dma_start(ssc_s[:], ssc)

            # Q/K projections -> raw Q1,Q2,K1,K2 [128, T], kt-outer so
            # matmuls start as soon as each xT chunk lands
            raw = {}
            for name, w_s, b_s in (("q", wqs_list[0], bq_s),
                                   ("k", wqs_list[1], bk_s)):
                pss = []
                for _i in range(8):
                    pst = ps_proj.tile([P, 512], F32, tag="ps", name=f"ps{_i}")
                    pss.append(pst)
                for kt in range(KT):
                    for m in range(2):
                        for n in range(T // 512):
                            nc.tensor.matmul(
                                pss[m * 4 + n][:],
                                w_s[:, kt, m * P:(m + 1) * P],
                                xts[kt][:, n * 512:(n + 1) * 512],
                                start=(kt == 0), stop=(kt == KT - 1),
                            )
                for m in range(2):
                    rt = qk.tile([P, T], BF16, tag=f"raw{name}{m}")
                    raw[(name, m)] = rt
                    for n in range(T // 512):
                        nc.scalar.activation(
                            out=rt[:, n * 512:(n + 1) * 512],
                            in_=pss[m * 4 + n][:],
                            func=Ident, bias=b_s[:, m:m + 1], scale=1.0,
                        )

            # RoPE (sqrt of the 1/sqrt(HD) score scale folded into csc/ssc)
            rot = {}
            for name in ("q", "k"):
                x1, x2 = raw[(name, 0)], raw[(name, 1)]
                t1 = rtmp.tile([P, T], BF16, tag="rt")
                nc.vector.tensor_mul(t1[:], x1[:], csc_s[:])
                t2 = rtmp.tile([P, T], BF16, tag="rt")
                nc.vector.tensor_mul(t2[:], x2[:], ssc_s[:])
                t3 = rtmp.tile([P, T], BF16, tag="rt")
                nc.vector.tensor_mul(t3[:], x1[:], ssc_s[:])
                y1 = qk.tile([P, T], BF16, tag=f"raw{name}0")
                nc.vector.tensor_sub(y1[:], t1[:], t2[:])
                t4 = rtmp.tile([P, T], BF16, tag="rt")
                nc.vector.tensor_mul(t4[:], x2[:], csc_s[:])
                y2 = qk.tile([P, T], BF16, tag=f"raw{name}1")
                nc.vector.tensor_add(y2[:], t3[:], t4[:])
                rot[name] = (y1, y2)

            # merge halves into head-interleaved tiles: tile j holds heads
            # (2j, 2j+1); head h occupies partitions 64*(h%2)..64*(h%2)+63
            # with [x1(32); x2(32)] inside.
            qc, kc = [], []
            for name in ("q", "k"):
                y1, y2 = rot[name]
                for j in range(2):
                    tgt = qk.tile([P, T], BF16, tag=f"c{name}{j}")
                    for i in range(2):
                        h = 2 * j + i
                        hs = slice(32 * h, 32 * h + 32)
                        nc.vector.tensor_copy(tgt[64 * i:64 * i + 32, :], y1[hs, :])
                        nc.vector.tensor_copy(tgt[64 * i + 32:64 * i + 64, :], y2[hs, :])
                    (qc if name == "q" else kc).append(tgt)

            # V projection -> [128, kt, h, 65] with ones column (kt-outer)
            wv_s = wpool.tile([P, KT, 2 * P], BF16, tag="w")
            nc.scalar.dma_start(wv_s[:], wv.rearrange("(ko ki) m -> ki ko m", ki=P))
            vss = []
            for _i in range(8):
                vst = ps_proj.tile([P, 2, 2 * P], F32, tag="ps", name=f"vps{_i}")
                vss.append(vst)
            for kt in range(KT):
                for tp in range(8):
                    for half in range(2):
                        t = 2 * tp + half
                        nc.tensor.matmul(
                            vss[tp][:, half, :],
                            xts[kt][:, t * P:(t + 1) * P],
                            wv_s[:, kt, :],
                            start=(kt == 0 and half == 0), stop=False,
                        )
            for tp in range(8):
                for half in range(2):
                    t = 2 * tp + half
                    nc.tensor.matmul(
                        vss[tp][:, half, :], ones_s[:], bv_s[:],
                        start=False, stop=(half == 1),
                    )
                for half in range(2):
                    t = 2 * tp + half
                    nc.scalar.activation(
                        out=v_s[:, t, :, 0:HD],
                        in_=vss[tp][:, half, :].rearrange("p (h d) -> p h d",
                                                          h=HPG),
                        func=Copy, scale=1.0,
                    )

        # ================= phase B: attention =================
        with ExitStack() as phb:
            expp = phb.enter_context(tc.tile_pool(name="exp", bufs=6))
            avsp = phb.enter_context(tc.tile_pool(name="avs", bufs=3))
            rp = phb.enter_context(tc.tile_pool(name="r", bufs=2))
            rrp = phb.enter_context(tc.tile_pool(name="rr", bufs=4))
            ps_sc = phb.enter_context(
                tc.tile_pool(name="pssc", bufs=2, space="PSUM"))
            ps_av = phb.enter_context(
                tc.tile_pool(name="psav", bufs=1, space="PSUM"))
            drp = phb.enter_context(
                tc.tile_pool(name="dr", bufs=2, space="DRAM"))
            wop = phb.enter_context(tc.tile_pool(name="wo", bufs=1))
            outb = phb.enter_context(tc.tile_pool(name="outb", bufs=3))

            wo_s = wop.tile([P, 2, D], F32R)
            nc.scalar.dma_start(
                wo_s[:], wo.rearrange("(ko ki) m -> ki ko m", ki=P))

            staged = {}

            def emit_normalize(qs, fine=False):
                avs, r_row = staged.pop(qs)
                d_r = drp.tile([HPG * QS], F32, tag="dr")
                nc.sync.dma_start(d_r[None, :],
                                  r_row.rearrange("o h q -> o (h q)"))
                rr = rrp.tile([HD, HPG, QS], F32, tag="rr")
                nc.sync.dma_start(
                    rr[:], d_r[None, :].broadcast_to([HD, HPG * QS])
                    .rearrange("p (h q) -> p h q", h=HPG))
                rrs = {h: rr[:, h, :] for h in range(HPG)}
                if not fine:
                    for tg in range(2):
                        for i in range(2):
                            h = 2 * tg + i
                            nc.vector.tensor_mul(
                                at_tiles[tg][i * HD:(i + 1) * HD,
                                             qs * QS:(qs + 1) * QS],
                                avs[0:HD, h, :], rrs[h][:],
                            )
                else:
                    # per-qt columns so outproj can chase the muls
                    for qt in range(4 * qs, 4 * qs + 4):
                        c0 = qt * P - qs * QS
                        for tg in range(2):
                            for i in range(2):
                                h = 2 * tg + i
                                nc.vector.tensor_mul(
                                    at_tiles[tg][i * HD:(i + 1) * HD,
                                                 qt * P:(qt + 1) * P],
                                    avs[0:HD, h, c0:c0 + P],
                                    rrs[h][:, c0:c0 + P],
                                )
                        emit_outproj_qt(qt)

            def emit_outproj_qt(qt):
                    ps = ps_sc.tile([P, 2, QS], F32, tag="sc")
                    ob = outb.tile([P, D], F32, tag="ob")
                    for nb in range(2):
                        for ktg in range(2):
                            nc.tensor.matmul(
                                ps[:, nb, :],
                                at_tiles[ktg][:, qt * P:(qt + 1) * P],
                                wo_s[:, ktg, nb * 512:(nb + 1) * 512],
                                start=(ktg == 0), stop=(ktg == 1),
                            )
                        nc.vector.tensor_copy(
                            out=ob[:, nb * 512:(nb + 1) * 512],
                            in_=ps[:, nb, :])
                    nc.sync.dma_start(out[qt * P:(qt + 1) * P, :], ob[:])

            def emit_outproj(qs):
                for qt in range(4 * qs, 4 * qs + 4):
                    emit_outproj_qt(qt)

            QORDER = [0, 1, 2, 3]
            for qidx, qs in enumerate(QORDER):
                av = ps_av.tile([HD + 1, HPG, QS], F32, tag="av")
                n_kt = 4 * qs + 4
                for kt in range(n_kt):
                    qoff = max(0, kt * P - qs * QS)
                    q0 = qs * QS + qoff
                    qext = QS - qoff
                    diag = kt * P >= qs * QS
                    for pair in range(2):
                        sc = ps_sc.tile([P, 2, QS], F32, tag="sc")
                        for i in range(2):
                            h = 2 * pair + i
                            hs = slice(64 * i, 64 * i + 64)
                            nc.tensor.matmul(
                                sc[:, i, qoff:QS],
                                kc[pair][hs, kt * P:(kt + 1) * P],
                                qc[pair][hs, q0:q0 + qext],
                                start=True, stop=True,
                                tile_position=(64 * i, 0),
                            )
                        ex = expp.tile([P, 2, QS], BF16, tag="ex")
                        nc.scalar.activation(
                            out=ex[:, :, qoff:QS], in_=sc[:, :, qoff:QS],
                            func=Exp, scale=1.0,
                        )
                        if diag:
                            # diagonal k-tile: zero strictly-upper corner (k > q)
                            nc.vector.tensor_mul(
                                ex[:, :, qoff:qoff + P],
                                ex[:, :, qoff:qoff + P],
                                m0_s[:, None, :].broadcast_to([P, 2, P]),
                            )
                        for i in range(2):
                            h = 2 * pair + i
                            nc.tensor.matmul(
                                av[:, h, qoff:QS],
                                _r(v_s[:, kt, h, :], cfg.mm_attn),
                                _r(ex[:, i, qoff:QS], cfg.mm_attn),
                                start=(kt == 0), stop=(kt == n_kt - 1),
                            )
                # r = exp(-ln(sums)) straight off the psum sums row (ACT)
                ln_row = rp.tile([1, HPG, QS], F32, tag="ln")
                nc.scalar.activation(out=ln_row[:], in_=av[HD:HD + 1, :, :],
                                     func=Ln, scale=1.0)
                r_row = rp.tile([1, HPG, QS], F32, tag="r")
                nc.scalar.activation(out=r_row[:], in_=ln_row[:],
                                     func=Exp, scale=-1.0)
                # stage AV psum to SBUF (frees psum fast)
                avs = avsp.tile([HD, HPG, QS], F32, tag="avs")
                nc.scalar.activation(out=avs[:], in_=av[0:HD, :, :],
                                     func=Copy, scale=1.0)
                staged[qs] = (avs, r_row)

                if qidx > 0:
                    emit_normalize(QORDER[qidx - 1])
                    emit_outproj(QORDER[qidx - 1])


            emit_normalize(QORDER[-1], fine=True)



# ---------------- host-side prep ----------------

def _perm(g):
    perm = []
    for half in range(2):
        for hh in range(HPG):
            for i in range(32):
                perm.append(256 * g + 64 * hh + 2 * i + half)
    return np.array(perm)


def host_inputs(inputs, c):
    b, g = c // 4, c % 4
    x, cos, sin = inputs["x"], inputs["cos"], inputs["sin"]
    Wq, bq, Wk, bk = inputs["Wq"], inputs["bq"], inputs["Wk"], inputs["bk"]
    Wv, bv, Wo = inputs["Wv"], inputs["bv"], inputs["Wo"]
    perm = _perm(g)
    s = math.sqrt(1.0 / math.sqrt(HD))
    cosT = np.ascontiguousarray(cos[0, 0].T) * s    # [32, T]
    sinT = np.ascontiguousarray(sin[0, 0].T) * s
    f32 = np.float32
    import ml_dtypes
    bf16 = ml_dtypes.bfloat16
    return {
        "xT": np.ascontiguousarray(x[b].T).astype(bf16),
        "wq": np.ascontiguousarray(Wq[perm, :].T).astype(bf16),
        "wk": np.ascontiguousarray(Wk[perm, :].T).astype(bf16),
        "wv": np.ascontiguousarray(Wv[256 * g:256 * (g + 1), :].T).astype(bf16),
        "wo": np.ascontiguousarray(Wo[:, 256 * g:256 * (g + 1)].T).astype(f32),
        "csc": np.ascontiguousarray(np.tile(cosT, (4, 1))).astype(bf16),
        "ssc": np.ascontiguousarray(np.tile(sinT, (4, 1))).astype(bf16),
        "bq2": np.ascontiguousarray(bq[perm].reshape(2, P).T).astype(f32),
        "bk2": np.ascontiguousarray(bk[perm].reshape(2, P).T).astype(f32),
        "bvr": np.ascontiguousarray(
            bv[256 * g:256 * (g + 1)].reshape(1, 2 * P)).astype(bf16),
        "m0": np.ascontiguousarray(
            (np.arange(P)[None, :] >= np.arange(P)[:, None])).astype(bf16),
        "ones_in": np.ones((P, P), bf16),
    }


def host_gather(results, bo):
    out = np.zeros((B, T, D), np.float32)
    for c in range(8):
        out[c // 4] += results[c]["out"]
    out += bo[None, None, :]
    return out


_NC_CACHE = {}


def _get_nc():
    if "nc" not in _NC_CACHE:
        _NC_CACHE["nc"] = build_nc(num_devices=8)
    return _NC_CACHE["nc"]


def kernel(**inputs):
    inputs = {k: np.asarray(v) for k, v in inputs.items()}
    nc = _get_nc()
    from concourse.bass_utils import run_bass_kernel_spmd
    in_maps = [host_inputs(inputs, c) for c in range(8)]
    res = run_bass_kernel_spmd(nc, in_maps, core_ids=list(range(8)))
    return host_gather(res.results, inputs["bo"].astype(np.float32))

